# revision 15
# baseline (speedup 1.0000x reference)
"""Trainium2 Bass kernel for nn_CausalGATLayer (N=512, IN=128, HID=256, OUT=256, 4 heads).

Strategy (8 NeuronCores, row-sharded: core c owns rows i in [c*64, (c+1)*64)):

  Head 2 (dominant cost): ce_w1 @ [V_i; V_j] = A_i + B_j, and since only the
  off-diagonal mean of `feat` is used and ce_w2 is linear:
    causal_mean = ce_w2 @ (sum_{i!=j} relu(A_i + B_j + ce_b1)) / (N^2-N) + ce_b2.
  Each core computes S_c = sum_{i in mine, all j} relu(A_i + B_j) - diag terms,
  a [256]-vector, as 128 (i, h-tile) units over B^T tiles [128h x 512j] bf16,
  split three ways:
    - DVE: tensor_scalar add+max (4x mode) into group scratch + one grouped
      tensor_scalar reduce (accum_out) per GRP units;
    - ACT: activation(Relu, bias, accum_out) reading B^T from PSUM (one
      self-contained instruction per unit);
    - Pool: fused scalar_tensor_tensor (relu vs a zeros tile) with accum_out.
  One 1KB AllReduce combines the 8 partial sums.

  Heads 1/3/4 are computed TRANSPOSED ([feat, row]) so hp_w/hp_b and
  hf_w1/hf_b1 fold into host-precomputed matrices (G1, G34, brow) and the
  fused MLP needs no PE transposes. Head-1 scores use host-folded
  w_dst = a_dst@W1 / w_src = a_src@W1 (one K=128 matmul each, no Wh copies).
  Head-3/4 row-degrees come from a PE ones-contraction of adjT;
  alpha3/alpha4 = adj/deg (row-constant scores).

  h2 is rank-1: its fused-MLP contribution folds into a single [256,256]
  matrix Mw applied to the AllReduced sum S, then one rank-1 PSUM update.
  Heads-chain ops are interleaved between unit groups so chain latency
  hides under unit throughput. LayerNorm rstd uses Newton rsqrt on DVE.
"""

import functools
import os
import numpy as np

_NO_CC = os.environ.get("K_NO_CC", "") == "1"   # debug/model: skip AllReduce

N = 512
D = 128      # IN_DIM
H = 256      # HID
HD = 64
OUTF = 256
NCORES = 8
CH = N // NCORES          # 64 rows per core
NT = N // 128             # 4 node tiles
P = 128

# head-2 unit split per h-tile (64 i-units each): ACT gets IA, Pool gets IP,
# DVE gets the rest (in groups of GRP with a grouped reduce).
IA = int(os.environ.get("K_IA", "16"))
IP = int(os.environ.get("K_IP", "20"))
GRP = int(os.environ.get("K_GRP", "8"))


def _build_program():
    import concourse.bass as bass
    import concourse.tile as tile
    from concourse import bacc, mybir
    from concourse.bass import ts

    f32 = mybir.dt.float32
    bf16 = mybir.dt.bfloat16
    i32 = mybir.dt.int32
    ALU = mybir.AluOpType
    ACTF = mybir.ActivationFunctionType

    nc = bacc.Bacc("TRN2", target_bir_lowering=False, debug=False,
                   num_devices=NCORES)

    def din(name, shape, dt=f32):
        return nc.dram_tensor(name, list(shape), dt, kind="ExternalInput")

    # ---- inputs (host prepacks layouts; see _prep_in_maps) ----
    cols_d = din("cols", (P, 6))             # ceB1|teB1|seB1 each [128,2] f32
    cw1RTb_d = din("cw1RTb", (D, H), bf16)   # ce_w1[:, 128:].T bf16
    VTb_d = din("VTb", (P, N + 2), bf16)     # V.T | wdst | wsrc bf16
    cw1LTb_d = din("cw1LTb", (D, H), bf16)   # ce_w1[:, :128].T bf16
    VcTb_d = din("VcTb", (P, CH), bf16)      # V[rows].T bf16
    # bf16 128-part pack: teW1Tab(256) seW1Tb(256) W1cb(64) wdstb(1) wsrcb(1)
    #                     w2tsb(256) hfW2Tb(512)
    B128B_LEN = 256 + 256 + 64 + 1 + 1 + 256 + 512 + 512
    b128b_d = din("b128b", (P, B128B_LEN), bf16)
    # bf16 64-part pack: pTb(512) teW1Tbb(256)
    b64b_d = din("b64b", (HD, 768), bf16)
    # f32 rows pack: dR(512) dC(64) teB2(64) seB2(64) brow(256) lnG(256) lnB(256)
    ROWS_LEN = 512 + 64 + 64 + 64 + 256 + 256 + 256
    rows_d = din("rows", (1, ROWS_LEN))
    # f32 128-part pack: adjT(4*64) Mw(2*256) G34(2*128)
    B128F_LEN = 256 + 256
    b128f_d = din("b128f", (P, B128F_LEN))
    g1_d = din("g1", (HD, OUTF + 2 * OUTF))  # G1 | ones x ln_g | ones x ln_b
    hfB2b_d = din("hfB2b", (1, OUTF), bf16)

    out_d = nc.dram_tensor("out", [CH, OUTF], f32, kind="ExternalOutput")

    with tile.TileContext(nc) as tc:
        cst = tc.alloc_tile_pool(name="cst", bufs=1)
        scr = tc.alloc_tile_pool(name="scr", bufs=2)
        grp = tc.alloc_tile_pool(name="grp", bufs=3)
        psbt = tc.alloc_tile_pool(name="psbt", bufs=1, space="PSUM")  # BT pinned
        psw = tc.alloc_tile_pool(name="psw", bufs=2, space="PSUM")    # [128,512]
        psb = tc.alloc_tile_pool(name="psb", bufs=2, space="PSUM")    # small
        psf = tc.alloc_tile_pool(name="psf", bufs=1, space="PSUM")    # pf1T
        dram = tc.alloc_tile_pool(name="dram", bufs=1, space="DRAM")

        # ---- critical-path DMAs ----
        cols = cst.tile([P, 6], f32, name="cols")
        nc.sync.dma_start(out=cols, in_=cols_d[:, :])
        ceB1 = cols[:, 0:2]
        teB1 = cols[:, 2:4]
        seB1 = cols[:, 4:6]
        cw1RTb = cst.tile([P, H], bf16, name="cw1RTb")
        nc.sync.dma_start(out=cw1RTb, in_=cw1RTb_d[:, :])
        VTbx = cst.tile([P, N + 2], bf16, name="VTbx")
        nc.sync.dma_start(out=VTbx, in_=VTb_d[:, :])
        VTb = VTbx[:, 0:N]
        wdstb = VTbx[:, N:N + 1]
        wsrcb = VTbx[:, N + 1:N + 2]
        cw1LTb = cst.tile([P, H], bf16, name="cw1LTb")
        nc.sync.dma_start(out=cw1LTb, in_=cw1LTb_d[:, :])
        VcTb = cst.tile([P, CH], bf16, name="VcTb")
        nc.sync.dma_start(out=VcTb, in_=VcTb_d[:, :])

        # ---- BT in PSUM (pinned; ACT units read it) + SBUF bf16 copies ----
        pbt0 = psbt.tile([P, N], f32, name="pbt0")
        pbt1 = psbt.tile([P, N], f32, name="pbt1")
        nc.tensor.matmul(pbt0, cw1RTb[:, ts(0, P)], VTb, start=True, stop=True)
        nc.tensor.matmul(pbt1, cw1RTb[:, ts(1, P)], VTb, start=True, stop=True)
        BT0 = cst.tile([P, N], bf16, name="BT0")
        nc.vector.tensor_copy(out=BT0, in_=pbt0)            # DVE
        BT1 = cst.tile([P, N], bf16, name="BT1")
        nc.scalar.activation(BT1, pbt1, ACTF.Copy)          # ACT

        # ---- A + bias, B diag columns ----
        ABcT = cst.tile([P, 2, CH], f32, name="ABcT")
        BcT = cst.tile([P, 2, CH], f32, name="BcT")
        for m in range(2):
            pab = psb.tile([P, CH], f32, name=f"pab{m}", tag="pab")
            nc.tensor.matmul(pab, cw1LTb[:, ts(m, P)], VcTb, start=True, stop=True)
            nc.vector.tensor_scalar(out=ABcT[:, m, :], in0=pab,
                                    scalar1=ceB1[:, m:m + 1], scalar2=None,
                                    op0=ALU.add)
            pbc = psb.tile([P, CH], f32, name=f"pbc{m}", tag="pab")
            nc.tensor.matmul(pbc, cw1RTb[:, ts(m, P)], VcTb, start=True, stop=True)
            nc.vector.tensor_copy(out=BcT[:, m, :], in_=pbc)

        # ---- remaining input DMAs (stream under the unit loop) ----
        rows = cst.tile([1, ROWS_LEN], f32, name="rows")
        nc.sync.dma_start(out=rows, in_=rows_d[:, :])
        o3 = [0]

        def rslice(ln):
            s = rows[:, o3[0]:o3[0] + ln]
            o3[0] += ln
            return s
        dR = rslice(N)
        dC = rslice(CH)
        teB2 = rslice(HD)
        seB2 = rslice(HD)
        brow = rslice(OUTF)
        lnG = rslice(OUTF)
        lnB = rslice(OUTF)

        b128f = cst.tile([P, B128F_LEN], f32, name="b128f")
        nc.sync.dma_start(out=b128f, in_=b128f_d[:, :])

        def adjT(t):                   # [:, t, :] of [128, 4, 64]
            return b128f[:, t * CH:(t + 1) * CH]

        def G34(m):                    # [:, m, :] of [128, 2, 128]
            return b128f[:, 256 + m * P:256 + (m + 1) * P]

        b128b = cst.tile([P, B128B_LEN], bf16, name="b128b")
        nc.sync.dma_start(out=b128b, in_=b128b_d[:, :])
        o = [0]

        def bslice(ln):
            s = b128b[:, o[0]:o[0] + ln]
            o[0] += ln
            return s
        teW1Tab = bslice(256)
        seW1Tb = bslice(256)
        W1cb = bslice(HD)
        bslice(2)  # (moved: wdst/wsrc ride in the VTb DMA)
        w2tsb_flat = bslice(256)
        hfW2Tb_flat = bslice(512)
        Mwb_flat = bslice(512)

        def Mwb(m):                    # [:, m, :] of [128, 2, 256]
            return Mwb_flat[:, m * OUTF:(m + 1) * OUTF]

        def w2tsb(m, c0, c1):          # [:, m, c0:c1] of [128, 2, 128]
            return w2tsb_flat[:, m * P + c0:m * P + c1]

        def hfW2Tb(m):                 # [:, m, :] of [128, 2, 256]
            return hfW2Tb_flat[:, m * OUTF:(m + 1) * OUTF]

        b64b = cst.tile([HD, 768], bf16, name="b64b")
        nc.sync.dma_start(out=b64b, in_=b64b_d[:, :])
        pTb = b64b[:, 0:512]
        teW1Tbb = b64b[:, 512:768]

        g1pack = cst.tile([HD, 3 * OUTF], f32, name="g1pack")
        nc.sync.dma_start(out=g1pack, in_=g1_d[:, :])
        G1 = g1pack[:, 0:OUTF]
        gB = g1pack[:, OUTF:3 * OUTF]
        hfB2b = cst.tile([1, OUTF], bf16, name="hfB2b")
        nc.sync.dma_start(out=hfB2b, in_=hfB2b_d[:, :])

        # ---- constants / scratch ----
        onesr = cst.tile([1, P], f32, name="onesr")
        nc.gpsimd.memset(onesr, 1.0)
        ones128c = cst.tile([P, 1], f32, name="ones128c")
        nc.gpsimd.memset(ones128c, 1.0)
        onesb = cst.tile([1, CH], bf16, name="onesb")
        nc.gpsimd.memset(onesb, 1.0)
        dummyA = cst.tile([P, N], bf16, name="dummyA")

        nDA = 64 - IA - IP
        nDG = (nDA + GRP - 1) // GRP if nDA > 0 else 0
        SaD = cst.tile([P, 2, max(nDG, 1)], f32, name="SaD")
        SaA = cst.tile([P, 2, max(IA, 1)], f32, name="SaA")
        nPG_ = (IP + GRP - 1) // GRP if IP > 0 else 0
        SaP = cst.tile([P, 2, max(nPG_, 1)], f32, name="SaP")

        # ---- persistent tiles for the heads chain ----
        tt0 = cst.tile([P, N], bf16, name="tt0")
        tt1 = cst.tile([P, N], bf16, name="tt1")
        st0 = cst.tile([P, N], bf16, name="st0")
        st1 = cst.tile([P, N], bf16, name="st1")
        tfsf = cst.tile([P, NT, P], f32, name="tfsf")      # [j, t, tf|sf]
        rhsU = cst.tile([P, NT, HD + 1], bf16, name="rhsU")
        eT = cst.tile([P, NT, CH], bf16, name="eT")
        adjTs = cst.tile([P, NT, CH], f32, name="adjTs")
        cRow = cst.tile([1, N], f32, name="cRow")
        rRow = cst.tile([1, CH], f32, name="rRow")
        degr = cst.tile([1, CH], f32, name="degr")
        srec = cst.tile([1, CH], f32, name="srec")
        U1nT = cst.tile([HD, CH], f32, name="U1nT")
        H1T = cst.tile([HD, CH], f32, name="H1T")
        H34T = cst.tile([P, CH], f32, name="H34T")
        Scol = cst.tile([P, 2], f32, name="Scol")
        f1T = cst.tile([P, 2, HD], bf16, name="f1T")

        pf1Tm = [psf.tile([P, HD], f32, name=f"pf1T{m}") for m in range(2)]

        # ================= heads chain steps (emitted interleaved) ==========
        steps = []

        def step(fn):
            steps.append(fn)
            return fn

        def s_tt0():
            ptt = psw.tile([P, N], f32, name="ptt", tag="pw")
            nc.tensor.matmul(ptt, teW1Tab[:, ts(0, P)], VTb, start=True, stop=False)
            nc.tensor.matmul(ptt, teW1Tbb[:, ts(0, P)], pTb, start=False, stop=True)
            nc.scalar.activation(tt0, ptt, ACTF.Relu, bias=teB1[:, 0:1])

        def s_tt1():
            ptt = psw.tile([P, N], f32, name="ptt2", tag="pw")
            nc.tensor.matmul(ptt, teW1Tab[:, ts(1, P)], VTb, start=True, stop=False)
            nc.tensor.matmul(ptt, teW1Tbb[:, ts(1, P)], pTb, start=False, stop=True)
            nc.scalar.activation(tt1, ptt, ACTF.Relu, bias=teB1[:, 1:2])

        def s_rows():
            pdst = psb.tile([1, N], f32, name="pdst", tag="pab")
            nc.tensor.matmul(pdst, wdstb, VTb, start=True, stop=True)
            nc.vector.scalar_tensor_tensor(out=cRow, in0=dR, scalar=0.1,
                                           in1=pdst, op0=ALU.mult, op1=ALU.add)
            psrc = psb.tile([1, CH], f32, name="psrc", tag="pab")
            nc.tensor.matmul(psrc, wsrcb, VcTb, start=True, stop=True)
            nc.vector.scalar_tensor_tensor(out=rRow, in0=dC, scalar=0.1,
                                           in1=psrc, op0=ALU.mult, op1=ALU.add)

        def s_st0():
            pst = psw.tile([P, N], f32, name="pst", tag="pw")
            nc.tensor.matmul(pst, seW1Tb[:, ts(0, P)], VTb, start=True, stop=True)
            nc.scalar.activation(st0, pst, ACTF.Relu, bias=seB1[:, 0:1])

        def s_st1():
            pst = psw.tile([P, N], f32, name="pst2", tag="pw")
            nc.tensor.matmul(pst, seW1Tb[:, ts(1, P)], VTb, start=True, stop=True)
            nc.scalar.activation(st1, pst, ACTF.Relu, bias=seB1[:, 1:2])

        def mk_rhsU(t):
            def s_rhsUt():
                pwc = psb.tile([P, HD], f32, name=f"pwc{t}", tag="pab")
                nc.tensor.matmul(pwc, VTb[:, ts(t, P)], W1cb, start=True, stop=True)
                nc.vector.tensor_copy(out=rhsU[:, t, 0:HD], in_=pwc)
                nc.gpsimd.memset(rhsU[:, t, HD:HD + 1], 1.0)
            return s_rhsUt

        scA = cst.tile([P, NT, CH], f32, name="scA")

        def mk_eT(t):
            def s_eTt():
                psc_t = psb.tile([P, CH], f32, name=f"psc{t}", tag="pab")
                nc.tensor.matmul(psc_t, cRow[:, ts(t, P)], onesr[:, 0:CH],
                                 start=True, stop=False)
                nc.tensor.matmul(psc_t, onesr, rRow, start=False, stop=True)
                nc.vector.tensor_copy(out=scA[:, t, :], in_=psc_t)
            return s_eTt

        def s_eTbatch():
            lk = scr.tile([P, NT * CH], f32, name="lk", tag="lk")
            nc.vector.scalar_tensor_tensor(out=lk, in0=scA[:, :, :],
                                           scalar=0.2, in1=scA[:, :, :],
                                           op0=ALU.mult, op1=ALU.max)
            ex = scr.tile([P, NT * CH], f32, name="ex", tag="ex")
            nc.scalar.activation(ex, lk, ACTF.Exp)
            nc.vector.tensor_tensor(out=eT[:, :, :], in0=ex,
                                    in1=b128f[:, 0:256], op=ALU.mult)

        def mk_tfsf(t):
            def s_tfsft():
                ptf = psb.tile([P, HD], f32, name=f"ptf{t}", tag="pab")
                nc.tensor.matmul(ptf, tt0[:, ts(t, P)], w2tsb(0, 0, HD),
                                 start=True, stop=False)
                nc.tensor.matmul(ptf, tt1[:, ts(t, P)], w2tsb(1, 0, HD),
                                 start=False, stop=False)
                nc.tensor.matmul(ptf, onesr, teB2, start=False, stop=True)
                nc.scalar.activation(tfsf[:, t, 0:HD], ptf, ACTF.Copy)
                pse = psb.tile([P, HD], f32, name=f"pse{t}", tag="pab")
                nc.tensor.matmul(pse, st0[:, ts(t, P)], w2tsb(0, HD, 2 * HD),
                                 start=True, stop=False)
                nc.tensor.matmul(pse, st1[:, ts(t, P)], w2tsb(1, HD, 2 * HD),
                                 start=False, stop=False)
                nc.tensor.matmul(pse, onesr, seB2, start=False, stop=True)
                nc.scalar.activation(tfsf[:, t, HD:P], pse, ACTF.Copy)
            return s_tfsft



        def s_aggU():
            pUT = psw.tile([HD + 1, CH], f32, name="pUT", tag="pw")
            for t in range(NT):
                nc.tensor.matmul(pUT, rhsU[:, t, :], eT[:, t, :],
                                 start=(t == 0), stop=(t == NT - 1))
            nc.vector.reciprocal(srec, pUT[HD:HD + 1, :])
            psr = psb.tile([HD, CH], f32, name="psr", tag="pab")
            nc.tensor.matmul(psr, onesr[:, 0:HD], srec, start=True, stop=True)
            srecB = scr.tile([HD, CH], f32, name="srecB", tag="srecB")
            nc.vector.tensor_copy(out=srecB, in_=psr)
            nc.vector.tensor_tensor(out=U1nT, in0=pUT[0:HD, :], in1=srecB,
                                    op=ALU.mult)
            tneg = scr.tile([HD, CH], f32, name="tneg", tag="tneg")
            nc.vector.tensor_scalar(out=tneg, in0=U1nT, scalar1=0.0,
                                    scalar2=None, op0=ALU.min)
            texp = scr.tile([HD, CH], f32, name="texp", tag="texp")
            nc.scalar.activation(texp, tneg, ACTF.Exp)
            tpos = scr.tile([HD, CH], f32, name="tpos", tag="tpos")
            nc.vector.tensor_scalar(out=tpos, in0=U1nT, scalar1=0.0,
                                    scalar2=None, op0=ALU.max)
            nc.vector.scalar_tensor_tensor(out=H1T, in0=texp, scalar=-1.0,
                                           in1=tpos, op0=ALU.add, op1=ALU.add)

        def s_deg():
            pdeg = psb.tile([1, CH], f32, name="pdeg", tag="pab")
            for t in range(NT):
                nc.tensor.matmul(pdeg, ones128c, adjT(t),
                                 start=(t == 0), stop=(t == NT - 1))
            nc.vector.reciprocal(degr, pdeg)
            pdb = psb.tile([P, CH], f32, name="pdb", tag="pab")
            nc.tensor.matmul(pdb, onesr, degr, start=True, stop=True)
            for t in range(NT):
                nc.vector.tensor_tensor(out=adjTs[:, t, :], in0=adjT(t),
                                        in1=pdb, op=ALU.mult)

        def s_agg34():
            p34 = psw.tile([P, CH], f32, name="p34", tag="pw")
            for t in range(NT):
                nc.tensor.matmul(p34, tfsf[:, t, :], adjTs[:, t, :],
                                 start=(t == 0), stop=(t == NT - 1))
            nc.vector.tensor_copy(out=H34T, in_=p34)

        def s_pf1T():
            for m in range(2):
                nc.tensor.matmul(pf1Tm[m], G1[:, ts(m, P)], H1T,
                                 start=True, stop=False)
                nc.tensor.matmul(pf1Tm[m], G34(m), H34T,
                                 start=False, stop=False)
                nc.tensor.matmul(pf1Tm[m], brow[:, ts(m, P)],
                                 onesr[:, 0:CH], start=False, stop=False)

        def s_diag():
            for m in range(2):
                dsum = scr.tile([P, CH], f32, name="dsum", tag="dsum")
                nc.vector.tensor_tensor(out=dsum, in0=ABcT[:, m, :],
                                        in1=BcT[:, m, :], op=ALU.add)
                dscr = scr.tile([P, CH], f32, name="dscr", tag="dscr")
                nc.vector.tensor_scalar(out=dscr, in0=dsum, scalar1=0.0,
                                        scalar2=None, op0=ALU.max, op1=ALU.add,
                                        accum_out=Scol[:, m:m + 1])

        steps = ([s_tt0, s_tt1, s_rows, s_st0, s_st1]
                 + [mk_rhsU(t) for t in range(NT)]
                 + [mk_eT(t) for t in range(NT)] + [s_eTbatch]
                 + [mk_tfsf(t) for t in range(NT)]
                 + [s_deg, s_aggU, s_agg34, s_pf1T, s_diag])

        # ================= unit groups =====================================
        BTs = (BT0, BT1)
        pbts = (pbt0, pbt1)

        def mk_dve(m, g):
            i0 = g * GRP
            i1 = min(i0 + GRP, nDA)
            w = i1 - i0

            def g_dve():
                gd = grp.tile([P, GRP, N], bf16, name="gd", tag="gd")
                for k in range(w):
                    nc.vector.tensor_scalar(out=gd[:, k, :], in0=BTs[m],
                                            scalar1=ABcT[:, m, i0 + k:i0 + k + 1],
                                            scalar2=0.0, op0=ALU.add,
                                            op1=ALU.max)
                nc.vector.tensor_scalar(out=gd[:, 0:w, :], in0=gd[:, 0:w, :],
                                        scalar1=0.0, scalar2=None, op0=ALU.add,
                                        op1=ALU.add,
                                        accum_out=SaD[:, m, g:g + 1])
            return g_dve

        def mk_act(m, u0, cnt):
            def g_act():
                for k in range(cnt):
                    i = nDA + u0 + k
                    nc.scalar.activation(dummyA, pbts[m], ACTF.Relu,
                                         bias=ABcT[:, m, i:i + 1],
                                         accum_out=SaA[:, m, u0 + k:u0 + k + 1])
            return g_act

        def mk_pool(m, g):
            i0 = g * GRP
            i1 = min(i0 + GRP, IP)
            w = i1 - i0

            def g_pool():
                gp = grp.tile([P, GRP, N], bf16, name="gp", tag="gp")
                for k in range(w):
                    i = nDA + IA + i0 + k
                    nc.gpsimd.tensor_scalar(out=gp[:, k, :], in0=BTs[m],
                                            scalar1=ABcT[:, m, i:i + 1],
                                            scalar2=0.0, op0=ALU.add,
                                            op1=ALU.max)
                nc.vector.tensor_scalar(out=gp[:, 0:w, :], in0=gp[:, 0:w, :],
                                        scalar1=0.0, scalar2=None, op0=ALU.add,
                                        op1=ALU.add,
                                        accum_out=SaP[:, m, g:g + 1])
            return g_pool

        ACT_CH = 4
        nPG = (IP + GRP - 1) // GRP if IP > 0 else 0
        dve_gs, act_gs, pool_gs = [], [], []
        for m in range(2):
            for g in range(nDG):
                dve_gs.append(mk_dve(m, g))
            for u0 in range(0, IA, ACT_CH):
                act_gs.append(mk_act(m, u0, min(ACT_CH, IA - u0)))
            for g in range(nPG):
                pool_gs.append(mk_pool(m, g))

        def rr(parts):
            out = []
            idx = [0] * len(parts)
            while any(i < len(p) for i, p in zip(idx, parts)):
                for k, p in enumerate(parts):
                    if idx[k] < len(p):
                        out.append(p[idx[k]])
                        idx[k] += 1
            return out

        groups = rr([dve_gs, act_gs, pool_gs])

        # ================= interleaved emission ============================
        ng, nst = len(groups), len(steps)
        gi, si = 0, 0
        while gi < ng or si < nst:
            # keep steps slightly ahead of proportional pace so the chain
            # finishes before the last unit groups
            want_step = si < nst and (gi >= ng or si * ng <= (gi + 2) * nst)
            if gi < ng and not want_step:
                groups[gi]()
                gi += 1
            elif si < nst:
                steps[si]()
                si += 1

        # ---- combine partial sums -> Scol (Scol holds +diag from s_diag) ----
        for m in range(2):
            parts = []
            if nDG > 0:
                rD = scr.tile([P, 1], f32, name="rD", tag="rD")
                nc.vector.tensor_reduce(rD, SaD[:, m, :], mybir.AxisListType.X,
                                        ALU.add)
                parts.append(rD)
            if IA > 0:
                rA = scr.tile([P, 1], f32, name="rA", tag="rA")
                nc.vector.tensor_reduce(rA, SaA[:, m, :], mybir.AxisListType.X,
                                        ALU.add)
                parts.append(rA)
            if IP > 0:
                rP = scr.tile([P, 1], f32, name="rP", tag="rP")
                nc.vector.tensor_reduce(rP, SaP[:, m, :], mybir.AxisListType.X,
                                        ALU.add)
                parts.append(rP)
            acc = parts[0]
            for j, nxt in enumerate(parts[1:]):
                na = scr.tile([P, 1], f32, name=f"na{j}", tag=f"na{j}")
                nc.vector.tensor_tensor(out=na, in0=acc, in1=nxt, op=ALU.add)
                acc = na
            nc.vector.tensor_tensor(out=Scol[:, m:m + 1], in0=acc,
                                    in1=Scol[:, m:m + 1], op=ALU.subtract)

        # ---- AllReduce of the [256] partial sum ----
        Sall = cst.tile([P, 2], f32, name="Sall")
        if _NO_CC:
            nc.vector.tensor_copy(out=Sall, in_=Scol)
        else:
            crs_in = dram.tile([H], f32, name="crs_in")
            crs_out = dram.tile([H], f32, name="crs_out")
            nc.sync.dma_start(out=crs_in[:].rearrange("(t p) -> p t", p=P),
                              in_=Scol)
            nc.gpsimd.collective_compute(
                "AllReduce", ALU.add,
                replica_groups=[list(range(NCORES))],
                ins=[crs_in.opt()],
                outs=[crs_out.opt()],
            )
            nc.sync.dma_start(out=Sall, in_=crs_out[:].rearrange(
                "(t p) -> p t", p=P))

        # ---- post-collective: w2h = S^T Mw ; pf1T += w2h x ones ----
        Sallb = cst.tile([P, 2], bf16, name="Sallb")
        nc.vector.tensor_copy(out=Sallb, in_=Sall)
        pw2h = psb.tile([1, OUTF], f32, name="pw2h", tag="pab")
        nc.tensor.matmul(pw2h, Sallb[:, 0:1], Mwb(0), start=True, stop=False)
        nc.tensor.matmul(pw2h, Sallb[:, 1:2], Mwb(1), start=False, stop=True)
        w2h = cst.tile([1, OUTF], f32, name="w2h")
        nc.vector.tensor_copy(out=w2h, in_=pw2h)
        nc.tensor.matmul(pf1Tm[0], w2h[:, ts(0, P)], onesr[:, 0:CH],
                         start=False, stop=True)
        nc.tensor.matmul(pf1Tm[1], w2h[:, ts(1, P)], onesr[:, 0:CH],
                         start=False, stop=True)

        # f1T = relu(pf1T) in bf16; pf2 = f1 @ hf_w2^T + hf_b2
        nc.scalar.activation(f1T[:, 0, :], pf1Tm[0], ACTF.Relu)
        nc.vector.tensor_scalar(out=f1T[:, 1, :], in0=pf1Tm[1], scalar1=0.0,
                                scalar2=None, op0=ALU.max)
        pf2 = psw.tile([CH, OUTF], f32, name="pf2", tag="pw")
        nc.tensor.matmul(pf2, f1T[:, 0, :], hfW2Tb(0), start=True,
                         stop=False)
        nc.tensor.matmul(pf2, f1T[:, 1, :], hfW2Tb(1), start=False,
                         stop=False)
        nc.tensor.matmul(pf2, onesb, hfB2b, start=False, stop=True)

        # ---- ELU + LayerNorm ----
        # LN is shift-invariant: use X' = min(exp(x),1) + relu(x) = elu(x)+1,
        # and var = E[X'^2] - mean(X')^2 (Square+accum on ACT runs parallel
        # to the mean/center chain on DVE).
        xexp = cst.tile([CH, OUTF], f32, name="xexp")
        nc.scalar.activation(xexp, pf2, ACTF.Exp)
        xpos = cst.tile([CH, OUTF], f32, name="xpos")
        nc.vector.tensor_scalar(out=xpos, in0=pf2, scalar1=0.0, scalar2=None,
                                op0=ALU.max)
        X = cst.tile([CH, OUTF], f32, name="X")
        nc.vector.scalar_tensor_tensor(out=X, in0=xexp, scalar=1.0, in1=xpos,
                                       op0=ALU.min, op1=ALU.add)
        sq2 = scr.tile([CH, OUTF], f32, name="sq2", tag="sq2")
        s2col = cst.tile([CH, 1], f32, name="s2col")
        nc.scalar.activation(sq2, X, ACTF.Square, accum_out=s2col[:, 0:1])
        musum = cst.tile([CH, 1], f32, name="musum")
        nc.vector.tensor_reduce(musum, X, mybir.AxisListType.X, ALU.add)
        mu = cst.tile([CH, 1], f32, name="mu")
        nc.vector.tensor_scalar(out=mu, in0=musum, scalar1=1.0 / OUTF,
                                scalar2=None, op0=ALU.mult)
        xc = cst.tile([CH, OUTF], f32, name="xc")
        nc.vector.tensor_scalar(out=xc, in0=X, scalar1=mu, scalar2=None,
                                op0=ALU.subtract)
        mu2 = cst.tile([CH, 1], f32, name="mu2")
        nc.vector.tensor_tensor(out=mu2, in0=mu, in1=mu, op=ALU.mult)
        v1 = cst.tile([CH, 1], f32, name="v1")
        nc.vector.tensor_scalar(out=v1, in0=s2col, scalar1=1.0 / OUTF,
                                scalar2=1e-5, op0=ALU.mult, op1=ALU.add)
        vcol = cst.tile([CH, 1], f32, name="vcol")
        nc.vector.tensor_tensor(out=vcol, in0=v1, in1=mu2, op=ALU.subtract)
        magic = cst.tile([CH, 1], i32, name="magic")
        nc.vector.memset(magic, 0x5f3759df)
        onei = cst.tile([CH, 1], i32, name="onei")
        nc.vector.memset(onei, 1)
        icol = cst.tile([CH, 1], i32, name="icol")
        nc.vector.tensor_tensor(out=icol, in0=vcol.bitcast(i32), in1=onei,
                                op=ALU.arith_shift_right)
        y0i = cst.tile([CH, 1], i32, name="y0i")
        nc.vector.tensor_tensor(out=y0i, in0=magic, in1=icol, op=ALU.subtract)
        hcol = cst.tile([CH, 1], f32, name="hcol")
        nc.vector.tensor_scalar(out=hcol, in0=vcol, scalar1=0.5, scalar2=None,
                                op0=ALU.mult)
        y = y0i.bitcast(f32)
        yt = [cst.tile([CH, 1], f32, name=f"yt{j}") for j in range(2)]
        for it in range(2):
            t1 = scr.tile([CH, 1], f32, name="nt1", tag="nt1")
            nc.vector.tensor_tensor(out=t1, in0=y, in1=y, op=ALU.mult)
            t2 = scr.tile([CH, 1], f32, name="nt2", tag="nt2")
            nc.vector.tensor_tensor(out=t2, in0=t1, in1=hcol, op=ALU.mult)
            t3 = scr.tile([CH, 1], f32, name="nt3", tag="nt3")
            nc.vector.tensor_scalar(out=t3, in0=t2, scalar1=-1.0, scalar2=1.5,
                                    op0=ALU.mult, op1=ALU.add)
            nc.vector.tensor_tensor(out=yt[it], in0=y, in1=t3, op=ALU.mult)
            y = yt[it]

        xg = scr.tile([CH, OUTF], f32, name="xg", tag="xg")
        nc.vector.scalar_tensor_tensor(out=xg, in0=xc, scalar=y,
                                       in1=gB[:, 0:OUTF], op0=ALU.mult,
                                       op1=ALU.mult)
        osb = cst.tile([CH, OUTF], f32, name="osb")
        nc.vector.tensor_tensor(out=osb, in0=xg, in1=gB[:, OUTF:2 * OUTF],
                                op=ALU.add)

        nc.sync.dma_start(out=out_d[:, :], in_=osb)

        for p in (dram, psf, psb, psw, psbt, grp, scr, cst):
            p.release()

    nc.compile()
    return nc


@functools.lru_cache(maxsize=1)
def _get_program():
    return _build_program()


def _prep_in_maps(inputs):
    import ml_dtypes
    f = np.float32
    bf = ml_dtypes.bfloat16
    V = np.ascontiguousarray(np.asarray(inputs["V"], f))
    adj = np.asarray(inputs["adj"]).astype(f)
    delta = np.asarray(inputs["delta"], f)
    prev = np.asarray(inputs["prev_hidden"], f)
    g = lambda k: np.asarray(inputs[k], f)

    W1 = g("W1"); ce_w1 = g("ce_w1"); te_w1 = g("te_w1"); se_w1 = g("se_w1")
    hp_w = g("hp_w"); hp_b = g("hp_b")
    hf_w1 = g("hf_w1"); hf_b1 = g("hf_b1")
    ce_w2 = g("ce_w2"); ce_b2 = g("ce_b2")
    VT = np.ascontiguousarray(V.T)

    wdst = (g("a_dst") @ W1)[0]           # (128,)
    wsrc = (g("a_src") @ W1)[0]

    W1blk = [np.ascontiguousarray(hf_w1[:, 64 * k:64 * (k + 1)].T)
             for k in range(4)]           # each (64, 256)
    G1 = hp_w[0].T @ W1blk[0]             # (64, 256)
    G34 = np.concatenate([hp_w[2].T @ W1blk[2], hp_w[3].T @ W1blk[3]], axis=0)
    Mw = (ce_w2.T @ hp_w[1].T @ W1blk[1]) / (N * N - N)   # (256S, 256o)
    crow = (ce_b2 @ hp_w[1].T + hp_b[1]) @ W1blk[1]       # (256,)
    brow = (hf_b1 + hp_b[0] @ W1blk[0] + hp_b[2] @ W1blk[2]
            + hp_b[3] @ W1blk[3] + crow)                  # (256,)

    w2ts = np.concatenate([g("te_w2").T, g("se_w2").T], axis=1)  # (256,128)
    w2tsb = w2ts.reshape(2, P, P).transpose(1, 0, 2).reshape(P, 2 * P)
    hfW2T = g("hf_w2").T                                          # (256,256)
    hfW2Tb = hfW2T.reshape(2, P, OUTF).transpose(1, 0, 2).reshape(P, 2 * OUTF)
    MwP = Mw.reshape(2, P, OUTF).transpose(1, 0, 2).reshape(P, 2 * OUTF)
    b128b = np.concatenate([
        np.ascontiguousarray(te_w1[:, :D].T),      # (128,256)
        np.ascontiguousarray(se_w1.T),             # (128,256)
        np.ascontiguousarray(W1.T[:, 0:HD]),       # (128,64)
        np.zeros((P, 2), np.float32),              # (moved to VTb DMA)
        w2tsb,
        hfW2Tb,
        MwP,
    ], axis=1).astype(bf)

    G34P = G34.reshape(P, 2 * P)   # (128f, 2m*128o) == [128,2,128] row-major

    cols6 = np.concatenate([g("ce_b1").reshape(2, P).T,
                            g("te_b1").reshape(2, P).T,
                            g("se_b1").reshape(2, P).T], axis=1)  # (128,6)

    b64b = np.concatenate([
        np.ascontiguousarray(prev.T),              # (64,512)
        np.ascontiguousarray(te_w1[:, D:].T),      # (64,256)
    ], axis=1).astype(bf)

    rows_common = np.concatenate([
        delta, delta[:CH], g("te_b2"), g("se_b2"), brow,
        g("ln_g"), g("ln_b"),
    ]).astype(f)

    common = {
        "cols": np.ascontiguousarray(cols6.astype(f)),
        "cw1RTb": np.ascontiguousarray(ce_w1[:, D:].T).astype(bf),
        "cw1LTb": np.ascontiguousarray(ce_w1[:, :D].T).astype(bf),
        "VTb": np.concatenate([VT, wdst[:, None], wsrc[:, None]],
                              axis=1).astype(bf),
        "b128b": np.ascontiguousarray(b128b),
        "b64b": np.ascontiguousarray(b64b),
        "g1": np.ascontiguousarray(np.concatenate(
            [G1, np.tile(g("ln_g")[None, :], (HD, 1)),
             np.tile(g("ln_b")[None, :], (HD, 1))], axis=1).astype(f)),
        "hfB2b": g("hf_b2")[None, :].astype(bf),
    }
    in_maps = []
    for c in range(NCORES):
        rsl = slice(c * CH, (c + 1) * CH)
        m = dict(common)
        VcT = np.ascontiguousarray(V[rsl].T)
        m["VcTb"] = VcT.astype(bf)
        adjTc = np.ascontiguousarray(adj[rsl].T)   # (512, 64)
        adjT_dev = adjTc.reshape(NT, P, CH).transpose(1, 0, 2).reshape(P, NT * CH)
        b128f = np.concatenate([adjT_dev, G34P], axis=1).astype(f)
        m["b128f"] = np.ascontiguousarray(b128f)
        r = rows_common.copy()
        r[N:N + CH] = delta[rsl]
        m["rows"] = r[None, :]
        in_maps.append(m)
    return in_maps


def _run(inputs, trace=False):
    from concourse.bass_utils import run_bass_kernel_spmd
    nc = _get_program()
    in_maps = _prep_in_maps(inputs)
    res = run_bass_kernel_spmd(nc, in_maps, list(range(NCORES)), trace=trace)
    out = np.concatenate([res.results[c]["out"] for c in range(NCORES)], axis=0)
    return out.astype(np.float32), res


def kernel(**inputs) -> np.ndarray:
    out, _ = _run(inputs)
    return out


# revision 16
# speedup vs baseline: 1.1363x; 1.1363x over previous
"""Trainium2 Bass kernel for nn_CausalGATLayer (N=512, IN=128, HID=256, OUT=256, 4 heads).

Strategy (8 NeuronCores, row-sharded: core c owns rows i in [c*64, (c+1)*64)):

  Head 2 (dominant cost): ce_w1 @ [V_i; V_j] = A_i + B_j, and since only the
  off-diagonal mean of `feat` is used and ce_w2 is linear:
    causal_mean = ce_w2 @ (sum_{i!=j} relu(A_i + B_j + ce_b1)) / (N^2-N) + ce_b2.
  Each core computes S_c = sum_{i in mine, all j} relu(A_i + B_j) - diag terms,
  a [256]-vector, as 128 (i, h-tile) units over B^T tiles [128h x 512j] bf16,
  split three ways:
    - DVE: tensor_scalar add+max (4x mode) into group scratch + one grouped
      tensor_scalar reduce (accum_out) per GRP units;
    - ACT: activation(Relu, bias, accum_out) reading B^T from PSUM (one
      self-contained instruction per unit);
    - Pool: fused scalar_tensor_tensor (relu vs a zeros tile) with accum_out.
  One 1KB AllReduce combines the 8 partial sums.

  Heads 1/3/4 are computed TRANSPOSED ([feat, row]) so hp_w/hp_b and
  hf_w1/hf_b1 fold into host-precomputed matrices (G1, G34, brow) and the
  fused MLP needs no PE transposes. Head-1 scores use host-folded
  w_dst = a_dst@W1 / w_src = a_src@W1 (one K=128 matmul each, no Wh copies).
  Head-3/4 row-degrees come from a PE ones-contraction of adjT;
  alpha3/alpha4 = adj/deg (row-constant scores).

  h2 is rank-1: its fused-MLP contribution folds into a single [256,256]
  matrix Mw applied to the AllReduced sum S, then one rank-1 PSUM update.
  Heads-chain ops are interleaved between unit groups so chain latency
  hides under unit throughput. LayerNorm rstd uses Newton rsqrt on DVE.
"""

import functools
import os
import numpy as np

_NO_CC = os.environ.get("K_NO_CC", "") == "1"   # debug/model: skip AllReduce

N = 512
D = 128      # IN_DIM
H = 256      # HID
HD = 64
OUTF = 256
NCORES = 8
CH = N // NCORES          # 64 rows per core
NT = N // 128             # 4 node tiles
P = 128

# head-2 unit split per h-tile (64 i-units each): ACT gets IA, Pool gets IP,
# DVE gets the rest (in groups of GRP with a grouped reduce).
IA = int(os.environ.get("K_IA", "16"))
IP = int(os.environ.get("K_IP", "20"))
GRP = int(os.environ.get("K_GRP", "8"))


def _build_program():
    import concourse.bass as bass
    import concourse.tile as tile
    from concourse import bacc, mybir
    from concourse.bass import ts

    f32 = mybir.dt.float32
    bf16 = mybir.dt.bfloat16
    i32 = mybir.dt.int32
    ALU = mybir.AluOpType
    ACTF = mybir.ActivationFunctionType

    nc = bacc.Bacc("TRN2", target_bir_lowering=False, debug=False,
                   num_devices=NCORES)

    def din(name, shape, dt=f32):
        return nc.dram_tensor(name, list(shape), dt, kind="ExternalInput")

    # ---- inputs (host prepacks layouts; see _prep_in_maps) ----
    cols_d = din("cols", (P, 6))             # ceB1|teB1|seB1 each [128,2] f32
    cw1RTb_d = din("cw1RTb", (D, H), bf16)   # ce_w1[:, 128:].T bf16
    VTb_d = din("VTb", (P, N + 2), bf16)     # V.T | wdst | wsrc bf16
    cw1LTb_d = din("cw1LTb", (D, H), bf16)   # ce_w1[:, :128].T bf16
    VcTb_d = din("VcTb", (P, CH), bf16)      # V[rows].T bf16
    # bf16 128-part pack: teW1Tab(256) seW1Tb(256) W1cb(64) wdstb(1) wsrcb(1)
    #                     w2tsb(256) hfW2Tb(512)
    B128B_LEN = 256 + 256 + 64 + 1 + 1 + 256 + 512 + 512
    b128b_d = din("b128b", (P, B128B_LEN), bf16)
    # bf16 64-part pack: pTb(512) teW1Tbb(256)
    b64b_d = din("b64b", (HD, 768), bf16)
    # f32 rows pack: dR(512) dC(64) teB2(64) seB2(64) brow(256) lnG(256) lnB(256)
    ROWS_LEN = 512 + 64 + 64 + 64 + 256 + 256 + 256
    rows_d = din("rows", (1, ROWS_LEN))
    # f32 128-part pack: adjT(4*64) Mw(2*256) G34(2*128)
    B128F_LEN = 256 + 256
    b128f_d = din("b128f", (P, B128F_LEN))
    g1_d = din("g1", (HD, OUTF + 2 * OUTF))  # G1 | ones x ln_g | ones x ln_b
    hfB2b_d = din("hfB2b", (1, OUTF), bf16)

    out_d = nc.dram_tensor("out", [CH, OUTF], f32, kind="ExternalOutput")

    with tile.TileContext(nc) as tc:
        cst = tc.alloc_tile_pool(name="cst", bufs=1)
        scr = tc.alloc_tile_pool(name="scr", bufs=2)
        grp = tc.alloc_tile_pool(name="grp", bufs=3)
        psbt = tc.alloc_tile_pool(name="psbt", bufs=1, space="PSUM")  # BT pinned
        psw = tc.alloc_tile_pool(name="psw", bufs=2, space="PSUM")    # [128,512]
        psb = tc.alloc_tile_pool(name="psb", bufs=2, space="PSUM")    # small
        psf = tc.alloc_tile_pool(name="psf", bufs=1, space="PSUM")    # pf1T
        dram = tc.alloc_tile_pool(name="dram", bufs=1, space="DRAM")

        # ---- critical-path DMAs ----
        cols = cst.tile([P, 6], f32, name="cols")
        nc.sync.dma_start(out=cols, in_=cols_d[:, :])
        ceB1 = cols[:, 0:2]
        teB1 = cols[:, 2:4]
        seB1 = cols[:, 4:6]
        cw1RTb = cst.tile([P, H], bf16, name="cw1RTb")
        nc.sync.dma_start(out=cw1RTb, in_=cw1RTb_d[:, :])
        VTbx = cst.tile([P, N + 2], bf16, name="VTbx")
        nc.sync.dma_start(out=VTbx, in_=VTb_d[:, :])
        VTb = VTbx[:, 0:N]
        wdstb = VTbx[:, N:N + 1]
        wsrcb = VTbx[:, N + 1:N + 2]
        cw1LTb = cst.tile([P, H], bf16, name="cw1LTb")
        nc.sync.dma_start(out=cw1LTb, in_=cw1LTb_d[:, :])
        VcTb = cst.tile([P, CH], bf16, name="VcTb")
        nc.sync.dma_start(out=VcTb, in_=VcTb_d[:, :])

        # ---- BT in PSUM (pinned; ACT units read it) + SBUF bf16 copies ----
        pbt0 = psbt.tile([P, N], f32, name="pbt0")
        pbt1 = psbt.tile([P, N], f32, name="pbt1")
        nc.tensor.matmul(pbt0, cw1RTb[:, ts(0, P)], VTb, start=True, stop=True)
        nc.tensor.matmul(pbt1, cw1RTb[:, ts(1, P)], VTb, start=True, stop=True)
        BT0 = cst.tile([P, N], bf16, name="BT0")
        nc.vector.tensor_copy(out=BT0, in_=pbt0)            # DVE
        BT1 = cst.tile([P, N], bf16, name="BT1")
        nc.scalar.activation(BT1, pbt1, ACTF.Copy)          # ACT

        # ---- A + bias, B diag columns ----
        ABcT = cst.tile([P, 2, CH], f32, name="ABcT")
        BcT = cst.tile([P, 2, CH], f32, name="BcT")
        for m in range(2):
            pab = psb.tile([P, CH], f32, name=f"pab{m}", tag="pab")
            nc.tensor.matmul(pab, cw1LTb[:, ts(m, P)], VcTb, start=True, stop=True)
            nc.vector.tensor_scalar(out=ABcT[:, m, :], in0=pab,
                                    scalar1=ceB1[:, m:m + 1], scalar2=None,
                                    op0=ALU.add)
            pbc = psb.tile([P, CH], f32, name=f"pbc{m}", tag="pab")
            nc.tensor.matmul(pbc, cw1RTb[:, ts(m, P)], VcTb, start=True, stop=True)
            nc.vector.tensor_copy(out=BcT[:, m, :], in_=pbc)

        # ---- remaining input DMAs (stream under the unit loop) ----
        rows = cst.tile([1, ROWS_LEN], f32, name="rows")
        nc.sync.dma_start(out=rows, in_=rows_d[:, :])
        o3 = [0]

        def rslice(ln):
            s = rows[:, o3[0]:o3[0] + ln]
            o3[0] += ln
            return s
        dR = rslice(N)
        dC = rslice(CH)
        teB2 = rslice(HD)
        seB2 = rslice(HD)
        brow = rslice(OUTF)
        lnG = rslice(OUTF)
        lnB = rslice(OUTF)

        b128f = cst.tile([P, B128F_LEN], f32, name="b128f")
        nc.sync.dma_start(out=b128f, in_=b128f_d[:, :])

        def adjT(t):                   # [:, t, :] of [128, 4, 64]
            return b128f[:, t * CH:(t + 1) * CH]

        def G34(m):                    # [:, m, :] of [128, 2, 128]
            return b128f[:, 256 + m * P:256 + (m + 1) * P]

        b128b = cst.tile([P, B128B_LEN], bf16, name="b128b")
        nc.sync.dma_start(out=b128b, in_=b128b_d[:, :])
        o = [0]

        def bslice(ln):
            s = b128b[:, o[0]:o[0] + ln]
            o[0] += ln
            return s
        teW1Tab = bslice(256)
        seW1Tb = bslice(256)
        W1cb = bslice(HD)
        bslice(2)  # (moved: wdst/wsrc ride in the VTb DMA)
        w2tsb_flat = bslice(256)
        hfW2Tb_flat = bslice(512)
        Mwb_flat = bslice(512)

        def Mwb(m):                    # [:, m, :] of [128, 2, 256]
            return Mwb_flat[:, m * OUTF:(m + 1) * OUTF]

        def w2tsb(m, c0, c1):          # [:, m, c0:c1] of [128, 2, 128]
            return w2tsb_flat[:, m * P + c0:m * P + c1]

        def hfW2Tb(m):                 # [:, m, :] of [128, 2, 256]
            return hfW2Tb_flat[:, m * OUTF:(m + 1) * OUTF]

        b64b = cst.tile([HD, 768], bf16, name="b64b")
        nc.sync.dma_start(out=b64b, in_=b64b_d[:, :])
        pTb = b64b[:, 0:512]
        teW1Tbb = b64b[:, 512:768]

        g1pack = cst.tile([HD, 3 * OUTF], f32, name="g1pack")
        nc.sync.dma_start(out=g1pack, in_=g1_d[:, :])
        G1 = g1pack[:, 0:OUTF]
        gB = g1pack[:, OUTF:3 * OUTF]
        hfB2b = cst.tile([1, OUTF], bf16, name="hfB2b")
        nc.sync.dma_start(out=hfB2b, in_=hfB2b_d[:, :])

        # ---- constants / scratch ----
        onesr = cst.tile([1, P], f32, name="onesr")
        nc.gpsimd.memset(onesr, 1.0)
        ones128c = cst.tile([P, 1], f32, name="ones128c")
        nc.gpsimd.memset(ones128c, 1.0)
        onesb = cst.tile([1, CH], bf16, name="onesb")
        nc.gpsimd.memset(onesb, 1.0)
        dummyA = cst.tile([P, N], bf16, name="dummyA")

        nDA = 64 - IA - IP
        nDG = (nDA + GRP - 1) // GRP if nDA > 0 else 0
        SaD = cst.tile([P, 2, max(nDG, 1)], f32, name="SaD")
        SaA = cst.tile([P, 2, max(IA, 1)], f32, name="SaA")
        nPG_ = (IP + GRP - 1) // GRP if IP > 0 else 0
        SaP = cst.tile([P, 2, max(nPG_, 1)], f32, name="SaP")

        # ---- persistent tiles for the heads chain ----
        tt0 = cst.tile([P, N], bf16, name="tt0")
        tt1 = cst.tile([P, N], bf16, name="tt1")
        st0 = cst.tile([P, N], bf16, name="st0")
        st1 = cst.tile([P, N], bf16, name="st1")
        tfsf = cst.tile([P, NT, P], f32, name="tfsf")      # [j, t, tf|sf]
        rhsU = cst.tile([P, NT, HD + 1], bf16, name="rhsU")
        eT = cst.tile([P, NT, CH], bf16, name="eT")
        adjTs = cst.tile([P, NT, CH], f32, name="adjTs")
        cRow = cst.tile([1, N], f32, name="cRow")
        rRow = cst.tile([1, CH], f32, name="rRow")
        degr = cst.tile([1, CH], f32, name="degr")
        srec = cst.tile([1, CH], f32, name="srec")
        U1nT = cst.tile([HD, CH], f32, name="U1nT")
        H1T = cst.tile([HD, CH], f32, name="H1T")
        H34T = cst.tile([P, CH], f32, name="H34T")
        Scol = cst.tile([P, 2], f32, name="Scol")
        f1T = cst.tile([P, 2, HD], bf16, name="f1T")

        pf1Tm = [psf.tile([P, HD], f32, name=f"pf1T{m}") for m in range(2)]

        # ================= heads chain steps (emitted interleaved) ==========
        steps = []

        def step(fn):
            steps.append(fn)
            return fn

        def s_tt0():
            ptt = psw.tile([P, N], f32, name="ptt", tag="pw")
            nc.tensor.matmul(ptt, teW1Tab[:, ts(0, P)], VTb, start=True, stop=False)
            nc.tensor.matmul(ptt, teW1Tbb[:, ts(0, P)], pTb, start=False, stop=True)
            nc.scalar.activation(tt0, ptt, ACTF.Relu, bias=teB1[:, 0:1])

        def s_tt1():
            ptt = psw.tile([P, N], f32, name="ptt2", tag="pw")
            nc.tensor.matmul(ptt, teW1Tab[:, ts(1, P)], VTb, start=True, stop=False)
            nc.tensor.matmul(ptt, teW1Tbb[:, ts(1, P)], pTb, start=False, stop=True)
            nc.scalar.activation(tt1, ptt, ACTF.Relu, bias=teB1[:, 1:2])

        def s_rows():
            pdst = psb.tile([1, N], f32, name="pdst", tag="pab")
            nc.tensor.matmul(pdst, wdstb, VTb, start=True, stop=True)
            nc.vector.scalar_tensor_tensor(out=cRow, in0=dR, scalar=0.1,
                                           in1=pdst, op0=ALU.mult, op1=ALU.add)
            psrc = psb.tile([1, CH], f32, name="psrc", tag="pab")
            nc.tensor.matmul(psrc, wsrcb, VcTb, start=True, stop=True)
            nc.vector.scalar_tensor_tensor(out=rRow, in0=dC, scalar=0.1,
                                           in1=psrc, op0=ALU.mult, op1=ALU.add)

        def s_st0():
            pst = psw.tile([P, N], f32, name="pst", tag="pw")
            nc.tensor.matmul(pst, seW1Tb[:, ts(0, P)], VTb, start=True, stop=True)
            nc.scalar.activation(st0, pst, ACTF.Relu, bias=seB1[:, 0:1])

        def s_st1():
            pst = psw.tile([P, N], f32, name="pst2", tag="pw")
            nc.tensor.matmul(pst, seW1Tb[:, ts(1, P)], VTb, start=True, stop=True)
            nc.scalar.activation(st1, pst, ACTF.Relu, bias=seB1[:, 1:2])

        def mk_rhsU(t):
            def s_rhsUt():
                pwc = psb.tile([P, HD], f32, name=f"pwc{t}", tag="pab")
                nc.tensor.matmul(pwc, VTb[:, ts(t, P)], W1cb, start=True, stop=True)
                nc.vector.tensor_copy(out=rhsU[:, t, 0:HD], in_=pwc)
                nc.gpsimd.memset(rhsU[:, t, HD:HD + 1], 1.0)
            return s_rhsUt

        scA = cst.tile([P, NT, CH], f32, name="scA")

        def mk_eT(t):
            def s_eTt():
                psc_t = psb.tile([P, CH], f32, name=f"psc{t}", tag="pab")
                nc.tensor.matmul(psc_t, cRow[:, ts(t, P)], onesr[:, 0:CH],
                                 start=True, stop=False)
                nc.tensor.matmul(psc_t, onesr, rRow, start=False, stop=True)
                nc.vector.tensor_copy(out=scA[:, t, :], in_=psc_t)
            return s_eTt

        def s_eTbatch():
            lk = scr.tile([P, NT * CH], f32, name="lk", tag="lk")
            nc.vector.scalar_tensor_tensor(out=lk, in0=scA[:, :, :],
                                           scalar=0.2, in1=scA[:, :, :],
                                           op0=ALU.mult, op1=ALU.max)
            ex = scr.tile([P, NT * CH], f32, name="ex", tag="ex")
            nc.scalar.activation(ex, lk, ACTF.Exp)
            nc.vector.tensor_tensor(out=eT[:, :, :], in0=ex,
                                    in1=b128f[:, 0:256], op=ALU.mult)

        def mk_tfsf(t):
            def s_tfsft():
                ptf = psb.tile([P, HD], f32, name=f"ptf{t}", tag="pab")
                nc.tensor.matmul(ptf, tt0[:, ts(t, P)], w2tsb(0, 0, HD),
                                 start=True, stop=False)
                nc.tensor.matmul(ptf, tt1[:, ts(t, P)], w2tsb(1, 0, HD),
                                 start=False, stop=False)
                nc.tensor.matmul(ptf, onesr, teB2, start=False, stop=True)
                nc.scalar.activation(tfsf[:, t, 0:HD], ptf, ACTF.Copy)
                pse = psb.tile([P, HD], f32, name=f"pse{t}", tag="pab")
                nc.tensor.matmul(pse, st0[:, ts(t, P)], w2tsb(0, HD, 2 * HD),
                                 start=True, stop=False)
                nc.tensor.matmul(pse, st1[:, ts(t, P)], w2tsb(1, HD, 2 * HD),
                                 start=False, stop=False)
                nc.tensor.matmul(pse, onesr, seB2, start=False, stop=True)
                nc.scalar.activation(tfsf[:, t, HD:P], pse, ACTF.Copy)
            return s_tfsft



        def s_aggU():
            pUT = psw.tile([HD + 1, CH], f32, name="pUT", tag="pw")
            for t in range(NT):
                nc.tensor.matmul(pUT, rhsU[:, t, :], eT[:, t, :],
                                 start=(t == 0), stop=(t == NT - 1))
            nc.vector.reciprocal(srec, pUT[HD:HD + 1, :])
            psr = psb.tile([HD, CH], f32, name="psr", tag="pab")
            nc.tensor.matmul(psr, onesr[:, 0:HD], srec, start=True, stop=True)
            srecB = scr.tile([HD, CH], f32, name="srecB", tag="srecB")
            nc.vector.tensor_copy(out=srecB, in_=psr)
            nc.vector.tensor_tensor(out=U1nT, in0=pUT[0:HD, :], in1=srecB,
                                    op=ALU.mult)
            tneg = scr.tile([HD, CH], f32, name="tneg", tag="tneg")
            nc.vector.tensor_scalar(out=tneg, in0=U1nT, scalar1=0.0,
                                    scalar2=None, op0=ALU.min)
            texp = scr.tile([HD, CH], f32, name="texp", tag="texp")
            nc.scalar.activation(texp, tneg, ACTF.Exp)
            tpos = scr.tile([HD, CH], f32, name="tpos", tag="tpos")
            nc.vector.tensor_scalar(out=tpos, in0=U1nT, scalar1=0.0,
                                    scalar2=None, op0=ALU.max)
            nc.vector.scalar_tensor_tensor(out=H1T, in0=texp, scalar=-1.0,
                                           in1=tpos, op0=ALU.add, op1=ALU.add)

        def s_deg():
            pdeg = psb.tile([1, CH], f32, name="pdeg", tag="pab")
            for t in range(NT):
                nc.tensor.matmul(pdeg, ones128c, adjT(t),
                                 start=(t == 0), stop=(t == NT - 1))
            nc.vector.reciprocal(degr, pdeg)
            pdb = psb.tile([P, CH], f32, name="pdb", tag="pab")
            nc.tensor.matmul(pdb, onesr, degr, start=True, stop=True)
            for t in range(NT):
                nc.vector.tensor_tensor(out=adjTs[:, t, :], in0=adjT(t),
                                        in1=pdb, op=ALU.mult)

        def s_agg34():
            p34 = psw.tile([P, CH], f32, name="p34", tag="pw")
            for t in range(NT):
                nc.tensor.matmul(p34, tfsf[:, t, :], adjTs[:, t, :],
                                 start=(t == 0), stop=(t == NT - 1))
            nc.vector.tensor_copy(out=H34T, in_=p34)

        def s_pf1T():
            for m in range(2):
                nc.tensor.matmul(pf1Tm[m], G1[:, ts(m, P)], H1T,
                                 start=True, stop=False)
                nc.tensor.matmul(pf1Tm[m], G34(m), H34T,
                                 start=False, stop=False)
                nc.tensor.matmul(pf1Tm[m], brow[:, ts(m, P)],
                                 onesr[:, 0:CH], start=False, stop=False)

        def s_diag():
            for m in range(2):
                dsum = scr.tile([P, CH], f32, name="dsum", tag="dsum")
                nc.vector.tensor_tensor(out=dsum, in0=ABcT[:, m, :],
                                        in1=BcT[:, m, :], op=ALU.add)
                dscr = scr.tile([P, CH], f32, name="dscr", tag="dscr")
                nc.vector.tensor_scalar(out=dscr, in0=dsum, scalar1=0.0,
                                        scalar2=None, op0=ALU.max, op1=ALU.add,
                                        accum_out=Scol[:, m:m + 1])

        steps = ([s_tt0, s_tt1, s_rows, s_st0, s_st1]
                 + [mk_rhsU(t) for t in range(NT)]
                 + [mk_eT(t) for t in range(NT)] + [s_eTbatch]
                 + [mk_tfsf(t) for t in range(NT)]
                 + [s_deg, s_aggU, s_agg34, s_pf1T, s_diag])

        # ================= unit groups =====================================
        BTs = (BT0, BT1)
        pbts = (pbt0, pbt1)

        def mk_dve(m, g):
            i0 = g * GRP
            i1 = min(i0 + GRP, nDA)
            w = i1 - i0

            def g_dve():
                gd = grp.tile([P, GRP, N], bf16, name="gd", tag="gd")
                for k in range(w):
                    nc.vector.tensor_scalar(out=gd[:, k, :], in0=BTs[m],
                                            scalar1=ABcT[:, m, i0 + k:i0 + k + 1],
                                            scalar2=0.0, op0=ALU.add,
                                            op1=ALU.max)
                nc.vector.tensor_scalar(out=gd[:, 0:w, :], in0=gd[:, 0:w, :],
                                        scalar1=0.0, scalar2=None, op0=ALU.add,
                                        op1=ALU.add,
                                        accum_out=SaD[:, m, g:g + 1])
            return g_dve

        def mk_act(m, u0, cnt):
            def g_act():
                for k in range(cnt):
                    i = nDA + u0 + k
                    nc.scalar.activation(dummyA, pbts[m], ACTF.Relu,
                                         bias=ABcT[:, m, i:i + 1],
                                         accum_out=SaA[:, m, u0 + k:u0 + k + 1])
            return g_act

        # Pool groups: the gpsimd chunk is emitted now; its DVE reduce is
        # deferred (lagged) so it never blocks DVE's in-order stream while
        # the slow Pool chunk is still running.
        pool_pending = []
        POOL_LAG = 2

        def mk_pool(m, g):
            i0 = g * GRP
            i1 = min(i0 + GRP, IP)
            w = i1 - i0

            def g_pool():
                gp = grp.tile([P, GRP, N], bf16, name="gp", tag="gp")
                for k in range(w):
                    i = nDA + IA + i0 + k
                    nc.gpsimd.tensor_scalar(out=gp[:, k, :], in0=BTs[m],
                                            scalar1=ABcT[:, m, i:i + 1],
                                            scalar2=0.0, op0=ALU.add,
                                            op1=ALU.max)

                def red():
                    nc.vector.tensor_scalar(out=gp[:, 0:w, :],
                                            in0=gp[:, 0:w, :],
                                            scalar1=0.0, scalar2=None,
                                            op0=ALU.add, op1=ALU.add,
                                            accum_out=SaP[:, m, g:g + 1])
                pool_pending.append(red)
                if len(pool_pending) > POOL_LAG:
                    pool_pending.pop(0)()
            return g_pool

        ACT_CH = 4
        nPG = (IP + GRP - 1) // GRP if IP > 0 else 0
        dve_gs, act_gs, pool_gs = [], [], []
        for m in range(2):
            for g in range(nDG):
                dve_gs.append(mk_dve(m, g))
            for u0 in range(0, IA, ACT_CH):
                act_gs.append(mk_act(m, u0, min(ACT_CH, IA - u0)))
            for g in range(nPG):
                pool_gs.append(mk_pool(m, g))

        def rr(parts):
            out = []
            idx = [0] * len(parts)
            while any(i < len(p) for i, p in zip(idx, parts)):
                for k, p in enumerate(parts):
                    if idx[k] < len(p):
                        out.append(p[idx[k]])
                        idx[k] += 1
            return out

        groups = rr([dve_gs, act_gs, pool_gs])

        # ================= interleaved emission ============================
        ng, nst = len(groups), len(steps)
        gi, si = 0, 0
        while gi < ng or si < nst:
            # keep steps slightly ahead of proportional pace so the chain
            # finishes before the last unit groups
            want_step = si < nst and (gi >= ng or si * ng <= (gi + 2) * nst)
            if gi < ng and not want_step:
                groups[gi]()
                gi += 1
            elif si < nst:
                steps[si]()
                si += 1

        for red in pool_pending:
            red()
        pool_pending.clear()

        # ---- combine partial sums -> Scol (Scol holds +diag from s_diag) ----
        for m in range(2):
            parts = []
            if nDG > 0:
                rD = scr.tile([P, 1], f32, name="rD", tag="rD")
                nc.vector.tensor_reduce(rD, SaD[:, m, :], mybir.AxisListType.X,
                                        ALU.add)
                parts.append(rD)
            if IA > 0:
                rA = scr.tile([P, 1], f32, name="rA", tag="rA")
                nc.vector.tensor_reduce(rA, SaA[:, m, :], mybir.AxisListType.X,
                                        ALU.add)
                parts.append(rA)
            if IP > 0:
                rP = scr.tile([P, 1], f32, name="rP", tag="rP")
                nc.vector.tensor_reduce(rP, SaP[:, m, :], mybir.AxisListType.X,
                                        ALU.add)
                parts.append(rP)
            acc = parts[0]
            for j, nxt in enumerate(parts[1:]):
                na = scr.tile([P, 1], f32, name=f"na{j}", tag=f"na{j}")
                nc.vector.tensor_tensor(out=na, in0=acc, in1=nxt, op=ALU.add)
                acc = na
            nc.vector.tensor_tensor(out=Scol[:, m:m + 1], in0=acc,
                                    in1=Scol[:, m:m + 1], op=ALU.subtract)

        # ---- AllReduce of the [256] partial sum ----
        Sall = cst.tile([P, 2], f32, name="Sall")
        if _NO_CC:
            nc.vector.tensor_copy(out=Sall, in_=Scol)
        else:
            crs_in = dram.tile([H], f32, name="crs_in")
            crs_out = dram.tile([H], f32, name="crs_out")
            nc.sync.dma_start(out=crs_in[:].rearrange("(t p) -> p t", p=P),
                              in_=Scol)
            nc.gpsimd.collective_compute(
                "AllReduce", ALU.add,
                replica_groups=[list(range(NCORES))],
                ins=[crs_in.opt()],
                outs=[crs_out.opt()],
            )
            nc.sync.dma_start(out=Sall, in_=crs_out[:].rearrange(
                "(t p) -> p t", p=P))

        # ---- post-collective: w2h = S^T Mw ; pf1T += w2h x ones ----
        Sallb = cst.tile([P, 2], bf16, name="Sallb")
        nc.vector.tensor_copy(out=Sallb, in_=Sall)
        pw2h = psb.tile([1, OUTF], f32, name="pw2h", tag="pab")
        nc.tensor.matmul(pw2h, Sallb[:, 0:1], Mwb(0), start=True, stop=False)
        nc.tensor.matmul(pw2h, Sallb[:, 1:2], Mwb(1), start=False, stop=True)
        w2h = cst.tile([1, OUTF], f32, name="w2h")
        nc.vector.tensor_copy(out=w2h, in_=pw2h)
        nc.tensor.matmul(pf1Tm[0], w2h[:, ts(0, P)], onesr[:, 0:CH],
                         start=False, stop=True)
        nc.tensor.matmul(pf1Tm[1], w2h[:, ts(1, P)], onesr[:, 0:CH],
                         start=False, stop=True)

        # f1T = relu(pf1T) in bf16; pf2 = f1 @ hf_w2^T + hf_b2
        nc.scalar.activation(f1T[:, 0, :], pf1Tm[0], ACTF.Relu)
        nc.vector.tensor_scalar(out=f1T[:, 1, :], in0=pf1Tm[1], scalar1=0.0,
                                scalar2=None, op0=ALU.max)
        pf2 = psw.tile([CH, OUTF], f32, name="pf2", tag="pw")
        nc.tensor.matmul(pf2, f1T[:, 0, :], hfW2Tb(0), start=True,
                         stop=False)
        nc.tensor.matmul(pf2, f1T[:, 1, :], hfW2Tb(1), start=False,
                         stop=False)
        nc.tensor.matmul(pf2, onesb, hfB2b, start=False, stop=True)

        # ---- ELU + LayerNorm ----
        # LN is shift-invariant: use X' = min(exp(x),1) + relu(x) = elu(x)+1,
        # and var = E[X'^2] - mean(X')^2 (Square+accum on ACT runs parallel
        # to the mean/center chain on DVE).
        xexp = cst.tile([CH, OUTF], f32, name="xexp")
        nc.scalar.activation(xexp, pf2, ACTF.Exp)
        xpos = cst.tile([CH, OUTF], f32, name="xpos")
        nc.vector.tensor_scalar(out=xpos, in0=pf2, scalar1=0.0, scalar2=None,
                                op0=ALU.max)
        X = cst.tile([CH, OUTF], f32, name="X")
        nc.vector.scalar_tensor_tensor(out=X, in0=xexp, scalar=1.0, in1=xpos,
                                       op0=ALU.min, op1=ALU.add)
        sq2 = scr.tile([CH, OUTF], f32, name="sq2", tag="sq2")
        s2col = cst.tile([CH, 1], f32, name="s2col")
        nc.scalar.activation(sq2, X, ACTF.Square, accum_out=s2col[:, 0:1])
        musum = cst.tile([CH, 1], f32, name="musum")
        nc.vector.tensor_reduce(musum, X, mybir.AxisListType.X, ALU.add)
        mu = cst.tile([CH, 1], f32, name="mu")
        nc.vector.tensor_scalar(out=mu, in0=musum, scalar1=1.0 / OUTF,
                                scalar2=None, op0=ALU.mult)
        xc = cst.tile([CH, OUTF], f32, name="xc")
        nc.vector.tensor_scalar(out=xc, in0=X, scalar1=mu, scalar2=None,
                                op0=ALU.subtract)
        mu2 = cst.tile([CH, 1], f32, name="mu2")
        nc.vector.tensor_tensor(out=mu2, in0=mu, in1=mu, op=ALU.mult)
        v1 = cst.tile([CH, 1], f32, name="v1")
        nc.vector.tensor_scalar(out=v1, in0=s2col, scalar1=1.0 / OUTF,
                                scalar2=1e-5, op0=ALU.mult, op1=ALU.add)
        vcol = cst.tile([CH, 1], f32, name="vcol")
        nc.vector.tensor_tensor(out=vcol, in0=v1, in1=mu2, op=ALU.subtract)
        magic = cst.tile([CH, 1], i32, name="magic")
        nc.vector.memset(magic, 0x5f3759df)
        onei = cst.tile([CH, 1], i32, name="onei")
        nc.vector.memset(onei, 1)
        icol = cst.tile([CH, 1], i32, name="icol")
        nc.vector.tensor_tensor(out=icol, in0=vcol.bitcast(i32), in1=onei,
                                op=ALU.arith_shift_right)
        y0i = cst.tile([CH, 1], i32, name="y0i")
        nc.vector.tensor_tensor(out=y0i, in0=magic, in1=icol, op=ALU.subtract)
        hcol = cst.tile([CH, 1], f32, name="hcol")
        nc.vector.tensor_scalar(out=hcol, in0=vcol, scalar1=0.5, scalar2=None,
                                op0=ALU.mult)
        y = y0i.bitcast(f32)
        yt = [cst.tile([CH, 1], f32, name=f"yt{j}") for j in range(2)]
        for it in range(2):
            t1 = scr.tile([CH, 1], f32, name="nt1", tag="nt1")
            nc.vector.tensor_tensor(out=t1, in0=y, in1=y, op=ALU.mult)
            t2 = scr.tile([CH, 1], f32, name="nt2", tag="nt2")
            nc.vector.tensor_tensor(out=t2, in0=t1, in1=hcol, op=ALU.mult)
            t3 = scr.tile([CH, 1], f32, name="nt3", tag="nt3")
            nc.vector.tensor_scalar(out=t3, in0=t2, scalar1=-1.0, scalar2=1.5,
                                    op0=ALU.mult, op1=ALU.add)
            nc.vector.tensor_tensor(out=yt[it], in0=y, in1=t3, op=ALU.mult)
            y = yt[it]

        xg = scr.tile([CH, OUTF], f32, name="xg", tag="xg")
        nc.vector.scalar_tensor_tensor(out=xg, in0=xc, scalar=y,
                                       in1=gB[:, 0:OUTF], op0=ALU.mult,
                                       op1=ALU.mult)
        osb = cst.tile([CH, OUTF], f32, name="osb")
        nc.vector.tensor_tensor(out=osb, in0=xg, in1=gB[:, OUTF:2 * OUTF],
                                op=ALU.add)

        nc.sync.dma_start(out=out_d[:, :], in_=osb)

        for p in (dram, psf, psb, psw, psbt, grp, scr, cst):
            p.release()

    nc.compile()
    return nc


@functools.lru_cache(maxsize=1)
def _get_program():
    return _build_program()


def _prep_in_maps(inputs):
    import ml_dtypes
    f = np.float32
    bf = ml_dtypes.bfloat16
    V = np.ascontiguousarray(np.asarray(inputs["V"], f))
    adj = np.asarray(inputs["adj"]).astype(f)
    delta = np.asarray(inputs["delta"], f)
    prev = np.asarray(inputs["prev_hidden"], f)
    g = lambda k: np.asarray(inputs[k], f)

    W1 = g("W1"); ce_w1 = g("ce_w1"); te_w1 = g("te_w1"); se_w1 = g("se_w1")
    hp_w = g("hp_w"); hp_b = g("hp_b")
    hf_w1 = g("hf_w1"); hf_b1 = g("hf_b1")
    ce_w2 = g("ce_w2"); ce_b2 = g("ce_b2")
    VT = np.ascontiguousarray(V.T)

    wdst = (g("a_dst") @ W1)[0]           # (128,)
    wsrc = (g("a_src") @ W1)[0]

    W1blk = [np.ascontiguousarray(hf_w1[:, 64 * k:64 * (k + 1)].T)
             for k in range(4)]           # each (64, 256)
    G1 = hp_w[0].T @ W1blk[0]             # (64, 256)
    G34 = np.concatenate([hp_w[2].T @ W1blk[2], hp_w[3].T @ W1blk[3]], axis=0)
    Mw = (ce_w2.T @ hp_w[1].T @ W1blk[1]) / (N * N - N)   # (256S, 256o)
    crow = (ce_b2 @ hp_w[1].T + hp_b[1]) @ W1blk[1]       # (256,)
    brow = (hf_b1 + hp_b[0] @ W1blk[0] + hp_b[2] @ W1blk[2]
            + hp_b[3] @ W1blk[3] + crow)                  # (256,)

    w2ts = np.concatenate([g("te_w2").T, g("se_w2").T], axis=1)  # (256,128)
    w2tsb = w2ts.reshape(2, P, P).transpose(1, 0, 2).reshape(P, 2 * P)
    hfW2T = g("hf_w2").T                                          # (256,256)
    hfW2Tb = hfW2T.reshape(2, P, OUTF).transpose(1, 0, 2).reshape(P, 2 * OUTF)
    MwP = Mw.reshape(2, P, OUTF).transpose(1, 0, 2).reshape(P, 2 * OUTF)
    b128b = np.concatenate([
        np.ascontiguousarray(te_w1[:, :D].T),      # (128,256)
        np.ascontiguousarray(se_w1.T),             # (128,256)
        np.ascontiguousarray(W1.T[:, 0:HD]),       # (128,64)
        np.zeros((P, 2), np.float32),              # (moved to VTb DMA)
        w2tsb,
        hfW2Tb,
        MwP,
    ], axis=1).astype(bf)

    G34P = G34.reshape(P, 2 * P)   # (128f, 2m*128o) == [128,2,128] row-major

    cols6 = np.concatenate([g("ce_b1").reshape(2, P).T,
                            g("te_b1").reshape(2, P).T,
                            g("se_b1").reshape(2, P).T], axis=1)  # (128,6)

    b64b = np.concatenate([
        np.ascontiguousarray(prev.T),              # (64,512)
        np.ascontiguousarray(te_w1[:, D:].T),      # (64,256)
    ], axis=1).astype(bf)

    rows_common = np.concatenate([
        delta, delta[:CH], g("te_b2"), g("se_b2"), brow,
        g("ln_g"), g("ln_b"),
    ]).astype(f)

    common = {
        "cols": np.ascontiguousarray(cols6.astype(f)),
        "cw1RTb": np.ascontiguousarray(ce_w1[:, D:].T).astype(bf),
        "cw1LTb": np.ascontiguousarray(ce_w1[:, :D].T).astype(bf),
        "VTb": np.concatenate([VT, wdst[:, None], wsrc[:, None]],
                              axis=1).astype(bf),
        "b128b": np.ascontiguousarray(b128b),
        "b64b": np.ascontiguousarray(b64b),
        "g1": np.ascontiguousarray(np.concatenate(
            [G1, np.tile(g("ln_g")[None, :], (HD, 1)),
             np.tile(g("ln_b")[None, :], (HD, 1))], axis=1).astype(f)),
        "hfB2b": g("hf_b2")[None, :].astype(bf),
    }
    in_maps = []
    for c in range(NCORES):
        rsl = slice(c * CH, (c + 1) * CH)
        m = dict(common)
        VcT = np.ascontiguousarray(V[rsl].T)
        m["VcTb"] = VcT.astype(bf)
        adjTc = np.ascontiguousarray(adj[rsl].T)   # (512, 64)
        adjT_dev = adjTc.reshape(NT, P, CH).transpose(1, 0, 2).reshape(P, NT * CH)
        b128f = np.concatenate([adjT_dev, G34P], axis=1).astype(f)
        m["b128f"] = np.ascontiguousarray(b128f)
        r = rows_common.copy()
        r[N:N + CH] = delta[rsl]
        m["rows"] = r[None, :]
        in_maps.append(m)
    return in_maps


def _run(inputs, trace=False):
    from concourse.bass_utils import run_bass_kernel_spmd
    nc = _get_program()
    in_maps = _prep_in_maps(inputs)
    res = run_bass_kernel_spmd(nc, in_maps, list(range(NCORES)), trace=trace)
    out = np.concatenate([res.results[c]["out"] for c in range(NCORES)], axis=0)
    return out.astype(np.float32), res


def kernel(**inputs) -> np.ndarray:
    out, _ = _run(inputs)
    return out
